# revision 1
# baseline (speedup 1.0000x reference)
"""CaptioningRNN (LSTM + tiny spatial attention) Trainium2 kernel.

Contract: kernel(**inputs) takes FULL inputs (numpy), returns FULL output
(N, T, H) float32.  Internally: data-parallel over batch N across 8
NeuronCores (16 sequences per core, zero cross-core traffic).

Per-core algorithm:
  phase 0: P[t] = x_t @ Wx + b precomputed for all t (PE, bf16) -> DRAM
  phase 1: 512 sequential LSTM steps.
    state kept transposed: hT (h on partitions, n on free) feeds matmuls
    as the stationary operand directly; attention is computed in an
    (h-partition, (n,m)-free) layout so the h-contraction runs on the PE
    (ones-vector colsum) and the m-contraction is a segmented DVE reduce.
"""

import os
import sys
import numpy as np

sys.path.insert(0, "/opt/trn_rl_repo")

import ml_dtypes

BF16 = ml_dtypes.bfloat16

N, T, D, H, M = 128, 512, 512, 512, 16
NCORES = 8
NL = N // NCORES          # 16 sequences per core
KC = 4                    # 512 = 4 chunks of 128 (contraction dims)
J = 4 * H                 # 2048 gate columns
TB = 8                    # time steps per phase-0 row block
RB = NL * T // 128        # phase-0 row blocks (rows = n*TB + tt)

_CACHE = {}


def build(t_steps=T, has_bias=False, fuse_ttr=False):
    from concourse import bacc, mybir
    import concourse.tile as tile

    f32 = mybir.dt.float32
    bf16 = mybir.dt.bfloat16
    mult = mybir.AluOpType.mult
    add = mybir.AluOpType.add
    AF = mybir.ActivationFunctionType
    AX = mybir.AxisListType.X

    rb = NL * t_steps // 128

    nc = bacc.Bacc("TRN2", target_bir_lowering=False, debug=False,
                   num_devices=NCORES)

    # ---- I/O -----------------------------------------------------------
    xs = nc.dram_tensor("xs", [rb, 128, KC, 128], bf16, kind="ExternalInput")
    at_d = nc.dram_tensor("at", [128, KC, NL, M], bf16, kind="ExternalInput")
    wx_d = nc.dram_tensor("wx", [128, KC, J], bf16, kind="ExternalInput")
    wh_d = nc.dram_tensor("wh", [128, KC, J], bf16, kind="ExternalInput")
    wa_d = nc.dram_tensor("wa", [128, KC, J], bf16, kind="ExternalInput")
    h0_d = nc.dram_tensor("h0t", [128, KC, NL], bf16, kind="ExternalInput")
    c0_d = nc.dram_tensor("c0", [NL, H], f32, kind="ExternalInput")
    id_d = nc.dram_tensor("ident", [NL, NL], bf16, kind="ExternalInput")
    oc_d = nc.dram_tensor("ones_col", [128, 1], bf16, kind="ExternalInput")
    or_d = nc.dram_tensor("ones_row", [1, 128], bf16, kind="ExternalInput")
    if has_bias:
        b_d = nc.dram_tensor("bvec", [1, J], f32, kind="ExternalInput")
    p_d = nc.dram_tensor("pbuf", [rb, 128, J], bf16)
    out_d = nc.dram_tensor("out", [NL, t_steps, H], f32, kind="ExternalOutput")

    inv_sqrt_h = float(1.0 / np.sqrt(H))

    from contextlib import ExitStack
    with tile.TileContext(nc) as tc, ExitStack() as stack:
        # ---- persistent constants -------------------------------------
        cpool = stack.enter_context(tc.tile_pool(name="consts", bufs=1))
        wh_s = cpool.tile([128, KC, J], bf16)
        wa_s = cpool.tile([128, KC, J], bf16)
        at_s = cpool.tile([128, KC, NL, M], bf16)
        h0_s = cpool.tile([128, KC, NL], bf16)
        id_s = cpool.tile([NL, NL], bf16)
        oc_s = cpool.tile([128, 1], bf16)
        or_s = cpool.tile([1, 128], bf16)
        nc.sync.dma_start(out=wh_s[:, :, :], in_=wh_d.ap()[:, :, :])
        nc.sync.dma_start(out=wa_s[:, :, :], in_=wa_d.ap()[:, :, :])
        nc.sync.dma_start(out=at_s[:, :, :, :], in_=at_d.ap()[:, :, :, :])
        nc.sync.dma_start(out=h0_s[:, :, :], in_=h0_d.ap()[:, :, :])
        nc.sync.dma_start(out=id_s[:, :], in_=id_d.ap()[:, :])
        nc.sync.dma_start(out=oc_s[:, :], in_=oc_d.ap()[:, :])
        nc.sync.dma_start(out=or_s[:, :], in_=or_d.ap()[:, :])

        # ---- phase 0: P = x @ Wx (+ b) --------------------------------
        with tc.tile_pool(name="ph0", bufs=1) as p0c, \
             tc.tile_pool(name="ph0x", bufs=3) as p0x, \
             tc.tile_pool(name="ph0o", bufs=3) as p0o, \
             tc.tile_pool(name="ps0", bufs=2, space="PSUM") as ps0:
            wx_s = p0c.tile([128, KC, J], bf16)
            nc.sync.dma_start(out=wx_s[:, :, :], in_=wx_d.ap()[:, :, :])
            if has_bias:
                bf_s = p0c.tile([1, J], f32)
                nc.sync.dma_start(out=bf_s[:, :], in_=b_d.ap()[:, :])
                bb_s = p0c.tile([1, J], bf16)
                nc.vector.tensor_copy(bb_s[:, :], bf_s[:, :])
                psb = ps0.tile([128, J], f32, tag="psb")
                for jt in range(4):
                    nc.tensor.matmul(psb[:, jt * 512:(jt + 1) * 512],
                                     or_s[:, :],
                                     bb_s[:, jt * 512:(jt + 1) * 512],
                                     start=True, stop=True)
                brep = p0c.tile([128, J], bf16)
                nc.vector.tensor_copy(brep[:, :], psb[:, :])

            for b_i in range(rb):
                xt = p0x.tile([128, KC, 128], bf16, tag="xt")
                nc.sync.dma_start(out=xt[:, :, :], in_=xs.ap()[b_i, :, :, :])
                psp = ps0.tile([128, J], f32, tag="psp")
                for kc in range(KC):
                    for jt in range(4):
                        nc.tensor.matmul(
                            psp[:, jt * 512:(jt + 1) * 512],
                            xt[:, kc, :],
                            wx_s[:, kc, jt * 512:(jt + 1) * 512],
                            start=(kc == 0), stop=(kc == KC - 1))
                pout = p0o.tile([128, J], bf16, tag="pout")
                for jt in range(4):
                    sl = slice(jt * 512, (jt + 1) * 512)
                    if has_bias:
                        nc.vector.tensor_tensor(pout[:, sl], psp[:, sl],
                                                brep[:, sl], add)
                    elif jt in (1, 3):
                        nc.scalar.copy(pout[:, sl], psp[:, sl])
                    else:
                        nc.vector.tensor_copy(pout[:, sl], psp[:, sl])
                nc.sync.dma_start(out=p_d.ap()[b_i, :, :], in_=pout[:, :])

        # ---- phase 1: recurrence --------------------------------------
        with tc.tile_pool(name="state", bufs=2) as stp, \
             tc.tile_pool(name="work", bufs=2) as wk, \
             tc.tile_pool(name="pin", bufs=3) as pin, \
             tc.tile_pool(name="hout", bufs=3) as hop, \
             tc.tile_pool(name="ps_a", bufs=1, space="PSUM") as psa_p, \
             tc.tile_pool(name="ps_s", bufs=1, space="PSUM") as pss:

            c_t = stp.tile([NL, H], f32, tag="c")
            nc.sync.dma_start(out=c_t[:, :], in_=c0_d.ap()[:, :])
            hT = h0_s

            for t in range(t_steps):
                p_t = pin.tile([NL, J], bf16, tag="pt")
                b_i, tt = divmod(t, TB)
                nc.sync.dma_start(out=p_t[:, :],
                                  in_=p_d.ap()[b_i, tt * NL:(tt + 1) * NL, :])

                # -- attention: scores via elementwise + PE colsum
                s2 = wk.tile([128, KC, NL, M], bf16, tag="s2")
                for kc in range(KC):
                    nc.gpsimd.tensor_tensor(
                        s2[:, kc, :, :], at_s[:, kc, :, :],
                        hT[:, kc, :, None].broadcast_to([128, NL, M]), mult)
                psz = pss.tile([1, NL, M], f32, tag="z")
                for kc in range(KC):
                    nc.tensor.matmul(psz[:, :, :], oc_s[:, :], s2[:, kc, :, :],
                                     start=(kc == 0), stop=(kc == KC - 1))
                e_t = wk.tile([1, NL, M], bf16, tag="e")
                nc.scalar.activation(e_t[:, :, :], psz[:, :, :], AF.Exp,
                                     scale=inv_sqrt_h)
                sum_e = wk.tile([1, NL, 1], f32, tag="sume")
                nc.vector.tensor_reduce(sum_e[:, :, :], e_t[:, :, :], AX, add)
                rec = wk.tile([1, NL, 1], f32, tag="rec")
                nc.vector.reciprocal(rec[:, :, :], sum_e[:, :, :])
                w_t = wk.tile([1, NL, M], bf16, tag="wt")
                nc.vector.tensor_tensor(
                    w_t[:, :, :], e_t[:, :, :],
                    rec[:, :, :].broadcast_to([1, NL, M]), mult)
                pse = pss.tile([128, NL, M], f32, tag="erep")
                nc.tensor.matmul(pse[:, :, :], or_s[:, :], w_t[:, :, :],
                                 start=True, stop=True)
                w_b = wk.tile([128, NL, M], bf16, tag="eb")
                nc.vector.tensor_copy(w_b[:, :, :], pse[:, :, :])

                attnU = wk.tile([128, KC, NL, 1], f32, tag="attnU")
                p2 = wk.tile([128, KC, NL, M], bf16, tag="p2")
                if fuse_ttr:
                    for kc in range(KC):
                        nc.vector.tensor_tensor_reduce(
                            p2[:, kc, :, :], at_s[:, kc, :, :], w_b[:, :, :],
                            scale=1.0, scalar=0.0, op0=mult, op1=add,
                            accum_out=attnU[:, kc, :, :])
                else:
                    for kc in range(KC):
                        nc.vector.tensor_tensor(
                            p2[:, kc, :, :], at_s[:, kc, :, :], w_b[:, :, :],
                            mult)
                        nc.vector.tensor_reduce(
                            attnU[:, kc, :, :], p2[:, kc, :, :], AX, add)
                aT = wk.tile([128, KC, NL], bf16, tag="aT")
                nc.vector.tensor_copy(aT[:, :, :], attnU[:, :, :, 0])

                # -- gates: a = P_t + h@Wh + attn@Wattn  (PSUM accumulate)
                psa = psa_p.tile([NL, J], f32, tag="a")
                for jt in range(4):
                    nc.tensor.matmul(psa[:, jt * 512:(jt + 1) * 512],
                                     id_s[:, :],
                                     p_t[:, jt * 512:(jt + 1) * 512],
                                     start=True, stop=False)
                for kc in range(KC):
                    for jt in range(4):
                        nc.tensor.matmul(
                            psa[:, jt * 512:(jt + 1) * 512],
                            hT[:, kc, :],
                            wh_s[:, kc, jt * 512:(jt + 1) * 512],
                            start=False, stop=False)
                for kc in range(KC):
                    for jt in range(4):
                        nc.tensor.matmul(
                            psa[:, jt * 512:(jt + 1) * 512],
                            aT[:, kc, :],
                            wa_s[:, kc, jt * 512:(jt + 1) * 512],
                            start=False, stop=(kc == KC - 1))

                sig_i = wk.tile([NL, H], bf16, tag="si")
                sig_f = wk.tile([NL, H], bf16, tag="sf")
                sig_o = wk.tile([NL, H], bf16, tag="so")
                tan_g = wk.tile([NL, H], bf16, tag="tg")
                nc.scalar.activation(sig_f[:, :], psa[:, 512:1024], AF.Sigmoid)
                nc.scalar.activation(sig_i[:, :], psa[:, 0:512], AF.Sigmoid)
                nc.scalar.activation(tan_g[:, :], psa[:, 1536:2048], AF.Tanh)
                nc.scalar.activation(sig_o[:, :], psa[:, 1024:1536], AF.Sigmoid)

                t1 = wk.tile([NL, H], f32, tag="t1")
                nc.vector.tensor_tensor(t1[:, :], sig_f[:, :], c_t[:, :], mult)
                t2 = wk.tile([NL, H], bf16, tag="t2")
                nc.vector.tensor_tensor(t2[:, :], sig_i[:, :], tan_g[:, :],
                                        mult)
                c_n = stp.tile([NL, H], f32, tag="c")
                nc.vector.tensor_tensor(c_n[:, :], t1[:, :], t2[:, :], add)
                tan_c = wk.tile([NL, H], bf16, tag="tc")
                nc.scalar.activation(tan_c[:, :], c_n[:, :], AF.Tanh)
                h_bf = wk.tile([NL, H], bf16, tag="hbf")
                nc.vector.tensor_tensor(h_bf[:, :], sig_o[:, :], tan_c[:, :],
                                        mult)
                h_f = hop.tile([NL, H], f32, tag="hf")
                nc.vector.tensor_copy(h_f[:, :], h_bf[:, :])
                nc.sync.dma_start(out=out_d.ap()[:, t, :], in_=h_f[:, :])

                pst = pss.tile([128, KC * NL], bf16, tag="tr")
                for kc in range(KC):
                    nc.tensor.transpose(pst[:, kc * NL:(kc + 1) * NL],
                                        h_bf[:, kc * 128:(kc + 1) * 128],
                                        id_s[:, :])
                hT_n = stp.tile([128, KC, NL], bf16, tag="hT")
                nc.vector.tensor_copy(
                    hT_n[:, :, :],
                    pst[:, :].rearrange("p (kc nl) -> p kc nl", kc=KC))

                hT = hT_n
                c_t = c_n

    nc.compile()
    return nc


def _stage_inputs(x, A, Wx, Wh, Wattn, b, t_steps=T):
    """Shard + lay out inputs per core (host-side numpy staging)."""
    rb = NL * t_steps // 128
    h0 = A.mean(axis=(2, 3)).astype(np.float32)          # (N, H)
    ident = np.eye(NL, dtype=BF16)
    ones_col = np.ones((128, 1), dtype=BF16)
    ones_row = np.ones((1, 128), dtype=BF16)

    def wlay(w):
        return np.ascontiguousarray(
            w.astype(BF16).reshape(KC, 128, J).transpose(1, 0, 2))

    wxs, whs, was = wlay(Wx), wlay(Wh), wlay(Wattn)
    bvec = np.ascontiguousarray(b.astype(np.float32).reshape(1, J))

    maps = []
    for k in range(NCORES):
        ns = slice(k * NL, (k + 1) * NL)
        x_sh = x[ns, :t_steps].astype(BF16)              # (NL, t, D)
        # (tb, p, kc, n*TB+tt)
        xT = x_sh.transpose(2, 0, 1).reshape(KC, 128, NL, rb, TB)
        # row order within a block: r = tt*NL + n
        xs_st = np.ascontiguousarray(
            xT.transpose(3, 1, 0, 4, 2).reshape(rb, 128, KC, 128))
        A_sh = A[ns].reshape(NL, H, M).astype(BF16)
        at_st = np.ascontiguousarray(
            A_sh.transpose(1, 0, 2).reshape(KC, 128, NL, M)
            .transpose(1, 0, 2, 3))
        h0_sh = h0[ns]                                    # (NL, H)
        h0t = np.ascontiguousarray(
            h0_sh.T.astype(BF16).reshape(KC, 128, NL).transpose(1, 0, 2))
        m = {
            "xs": xs_st, "at": at_st, "wx": wxs, "wh": whs, "wa": was,
            "h0t": h0t, "c0": np.ascontiguousarray(h0_sh),
            "ident": ident, "ones_col": ones_col, "ones_row": ones_row,
        }
        if np.any(b != 0):
            m["bvec"] = bvec
        maps.append(m)
    return maps


def _get_nc(has_bias, t_steps=T):
    key = (has_bias, t_steps)
    if key not in _CACHE:
        _CACHE[key] = build(t_steps=t_steps, has_bias=has_bias)
    return _CACHE[key]


def run_cores(x, A, Wx, Wh, Wattn, b, t_steps=T, trace=False):
    from concourse.bass_utils import run_bass_kernel_spmd
    maps = _stage_inputs(x, A, Wx, Wh, Wattn, b, t_steps=t_steps)
    has_bias = "bvec" in maps[0]
    nc = _get_nc(has_bias, t_steps)
    res = run_bass_kernel_spmd(nc, maps, list(range(NCORES)), trace=trace)
    out = np.concatenate([res.results[k]["out"] for k in range(NCORES)],
                         axis=0)
    return np.asarray(out, dtype=np.float32), res


def kernel(x, A, Wx, Wh, Wattn, b):
    x = np.asarray(x, dtype=np.float32)
    A = np.asarray(A, dtype=np.float32)
    out, _ = run_cores(x, A,
                       np.asarray(Wx, dtype=np.float32),
                       np.asarray(Wh, dtype=np.float32),
                       np.asarray(Wattn, dtype=np.float32),
                       np.asarray(b, dtype=np.float32))
    return out

